# revision 36
# baseline (speedup 1.0000x reference)
# Multi-head attention (B=2, S=2048, D=1024, H=16, dh=64) on 8 TRN2 NeuronCores.
# Sharding: core = batch * 4 + head_group; each core handles one batch and 4 heads.
# Host prep: q/k/v packed per 512-token slab, feature-major, contiguous per
# partition (8KB DMA lines); weight slices packed likewise.
# Kernel (fused pipeline over token slabs, causal): projections for slab t ->
# attention for q-tile j=t over k-tiles 0..4t+3: scoresT = K^T@Q (2-head
# row-packed pairs, K=64, shared 2-bank psum tile) -> one exp per head-pair
# (ScalarE, scale fused) -> multiplicative keep-mask on the mixed staircase
# columns only (GpSimd, SBUF-SBUF) -> PV with ones-augmented V (M=65) giving
# softmax denominators for free -> unnormalized att evacuated to SBUF,
# normalization deferred: strided-partition reciprocal + K=1 broadcast matmuls
# into PSUM + DVE multiply reading PSUM directly -> row-parallel Wo with shared
# att weight loads -> bf16 outputs; partial outputs summed on host.
import numpy as np
import ml_dtypes

import concourse.bass as bass
import concourse.tile as tile
from concourse import bacc, mybir
from concourse import bass_utils

B, S, D = 2, 2048, 1024
H, DH = 16, 64
NCORES = 8
GROUPS = 4            # head groups per batch (cores per batch)
HPG = 4               # heads per group
FPG = HPG * DH        # 256 features per group
SQ_T, SK_T = 512, 128
NSQ, NSK = S // SQ_T, S // SK_T
NCH = D // 128        # 8 contraction chunks of d_model
BF16 = ml_dtypes.bfloat16

_BUILT = {}


def _classify(mask):
    """Per-tile classification in scoresT space: tile (i, j) covers
    k in [i*128, (i+1)*128), q in [j*512, (j+1)*512).

    Returns cls[(i, j)] in:
      "skip"                        -- no unmasked entries
      ("full", c0, c1, None)        -- all entries in [c0, c1) kept
      (uid, c0, c1, (m0, m1))       -- compute [c0, c1), mask-mul [m0, m1)
                                       with unique pattern uid
    plus the list of unique patterns (each [128, w] bf16, 1.0 = keep).
    """
    keep_t = (~np.asarray(mask, dtype=bool)).T  # [k, q], True = attend
    cls = {}
    pat_ids = {}
    patterns = []
    for j in range(NSQ):
        first = True
        for i in range(NSK):
            sub = keep_t[i * SK_T:(i + 1) * SK_T, j * SQ_T:(j + 1) * SQ_T]
            if not sub.any():
                cls[(i, j)] = "skip"
                continue
            cols = np.flatnonzero(sub.any(axis=0))
            c0, c1 = int(cols[0]), int(cols[-1]) + 1
            if first:
                c0, c1 = 0, SQ_T  # first kept tile must cover the psum bank
            first = False
            if sub.all():
                cls[(i, j)] = ("full", 0, SQ_T, None) if (c0, c1) == (0, SQ_T) \
                    else ("full", c0, c1, None)
                continue
            # columns inside the computed range that are not all-keep
            notfull = np.flatnonzero(~sub[:, c0:c1].all(axis=0)) + c0
            if len(notfull) == 0:
                cls[(i, j)] = ("full", c0, c1, None)
                continue
            m0, m1 = int(notfull[0]), int(notfull[-1]) + 1
            pat = np.ascontiguousarray(sub[:, m0:m1].astype(BF16))
            key = (pat.shape[1], pat.tobytes())
            if key not in pat_ids:
                pat_ids[key] = len(patterns)
                patterns.append(pat)
            cls[(i, j)] = (pat_ids[key], c0, c1, (m0, m1))
    return cls, patterns


def _build(cls, pat_widths):
    """pat_widths: list of unique keep-pattern widths (free size each)."""
    nc = bacc.Bacc("TRN2", target_bir_lowering=False, debug=False)
    dt = mybir.dt
    f32, bf = dt.float32, dt.bfloat16
    EXP = mybir.ActivationFunctionType.Exp
    KW = max(sum(pat_widths), 1)
    pat_off = np.cumsum([0] + list(pat_widths))

    xq = nc.dram_tensor("xqt", [NSQ, 128, NCH, SQ_T], bf, kind="ExternalInput").ap()
    xk = nc.dram_tensor("xkt", [NSQ, 128, NCH, SQ_T], bf, kind="ExternalInput").ap()
    xv = nc.dram_tensor("xvt", [NSQ, 128, NCH, SQ_T], bf, kind="ExternalInput").ap()
    wq = nc.dram_tensor("wqt", [128, NCH, FPG], bf, kind="ExternalInput").ap()
    wk = nc.dram_tensor("wkt", [128, NCH, FPG], bf, kind="ExternalInput").ap()
    wv = nc.dram_tensor("wvt", [128, NCH, FPG], bf, kind="ExternalInput").ap()
    wo = nc.dram_tensor("wot", [128, FPG // 128, D], bf, kind="ExternalInput").ap()
    kp = nc.dram_tensor("keep", [128, 2, KW], bf, kind="ExternalInput").ap()
    out = nc.dram_tensor("out", [S // 128, 128, D], bf, kind="ExternalOutput").ap()

    xq_v = xq.rearrange("t p c s -> p t c s")
    xk_v = xk.rearrange("t p c s -> p t c s")
    xv_v = xv.rearrange("t p c s -> p t c s")

    with tile.TileContext(nc) as tc:
        with (
            tc.tile_pool(name="consts", bufs=1) as consts,
            tc.tile_pool(name="x", bufs=2) as xpool,
            tc.tile_pool(name="sc", bufs=2, space="PSUM") as sc_ps,
            tc.tile_pool(name="pv", bufs=2, space="PSUM") as pv_ps,
            tc.tile_pool(name="pp", bufs=2, space="PSUM") as pp_ps,
            tc.tile_pool(name="work", bufs=2) as work,
            tc.tile_pool(name="probs", bufs=8) as prpool,
        ):
            wq_sb = consts.tile([128, NCH, FPG], bf)
            wk_sb = consts.tile([128, NCH, FPG], bf)
            wv_sb = consts.tile([128, NCH, FPG], bf)
            wo_sb = consts.tile([128, FPG // 128, D], bf)
            keep_sb = consts.tile([128, 2, KW], bf)
            ones_sb = consts.tile([128, 128], bf)
            qh_sb = consts.tile([128, 2, S], bf)
            # K stored zero-padded to full 128 contraction rows per head:
            # slot z=0 holds the even head's dims in rows 0:64 (rows 64:128
            # zero), z=1 the odd head's dims in rows 64:128 (rows 0:64 zero).
            # Scores then use full-row FWL-eligible weight loads; the zero
            # half contributes nothing, so results are exact.
            kh_sb = consts.tile([128, 2, 2, S], bf)
            vh_sb = consts.tile([128, NSK, HPG, DH + 1], bf)
            att_sb = consts.tile([128, 2, S], bf)
            l4 = consts.tile([128, SQ_T], f32)
            r4f = consts.tile([128, SQ_T], f32)
            r4 = consts.tile([128, SQ_T], bf)
            warm_rhs = consts.tile([128, SQ_T], bf)

            def warm_pe(n):
                """Dummy matmul burst: keeps/brings the PE HAM clock gate at
                2.4 GHz across windows where no real matmul is ready."""
                wps = sc_ps.tile([128, 2, SQ_T], f32, tag="sc")
                for w in range(n):
                    nc.tensor.matmul(wps[:, 0, :], ones_sb[:], warm_rhs[:],
                                     start=True, stop=True)
                # dummy reader (value irrelevant; later warm bursts reuse it)
                nc.vector.tensor_copy(warm_rhs[0:1, :], wps[0:1, 0, :])

            nc.vector.memset(ones_sb[:], 1.0)
            nc.vector.memset(warm_rhs[:], 1.0)
            nc.vector.memset(l4[:], 1.0)  # unused partitions -> r = 1
            nc.vector.memset(vh_sb[:, :, :, DH:DH + 1], 1.0)  # ones column
            nc.vector.memset(kh_sb[64:128, :, 0, :], 0.0)
            nc.vector.memset(kh_sb[0:64, :, 1, :], 0.0)

            x_tiles = {}

            def load_slab(t, split=False):
                xq_t = xpool.tile([128, NCH, SQ_T], bf, tag="xq")
                xk_t = xpool.tile([128, NCH, SQ_T], bf, tag="xk")
                xv_t = xpool.tile([128, NCH, SQ_T], bf, tag="xv")
                if split:
                    nc.sync.dma_start(wq_sb[:, 0:4, :], wq[:, 0:4, :])
                    nc.sync.dma_start(xq_t[:, 0:4, :], xq_v[:, t, 0:4, :])
                    nc.sync.dma_start(wq_sb[:, 4:8, :], wq[:, 4:8, :])
                    nc.sync.dma_start(xq_t[:, 4:8, :], xq_v[:, t, 4:8, :])
                    nc.sync.dma_start(wk_sb[:], wk)
                else:
                    nc.sync.dma_start(xq_t[:], xq_v[:, t])
                nc.scalar.dma_start(xk_t[:], xk_v[:, t])
                nc.sync.dma_start(xv_t[:], xv_v[:, t])
                x_tiles[t] = (xq_t, xk_t, xv_t)

            def proj_slab(t):
                sl = bass.ts(t, SQ_T)
                xq_t, xk_t, xv_t = x_tiles.pop(t)

                def qk_chain(wsb, xin, hout, hp):
                    hsl = bass.ts(hp, 128)
                    ps = pp_ps.tile([128, SQ_T], f32, tag="pp")
                    for c in range(NCH):
                        nc.tensor.matmul(ps[:], wsb[:, c, hsl], xin[:, c, :],
                                         start=(c == 0), stop=(c == NCH - 1))
                    if hout is kh_sb:
                        nc.vector.tensor_copy(kh_sb[0:64, hp, 0, sl],
                                              ps[0:64, :])
                        nc.vector.tensor_copy(kh_sb[64:128, hp, 1, sl],
                                              ps[64:128, :])
                    else:
                        nc.vector.tensor_copy(hout[:, hp, sl], ps[:])

                def v_chain(s4):
                    i = t * (SQ_T // SK_T) + s4
                    psv = pp_ps.tile([128, SQ_T], f32, tag="pp")
                    for c in range(NCH):
                        nc.tensor.matmul(psv[:, 0:FPG],
                                         xv_t[:, c, bass.ts(s4, SK_T)],
                                         wv_sb[:, c, :],
                                         start=(c == 0), stop=(c == NCH - 1))
                    nc.vector.tensor_copy(
                        vh_sb[:, i, :, 0:DH],
                        psv[:, 0:FPG].rearrange("p (h d) -> p h d", h=HPG))

                # hp0 q/k first so the first score matmul unblocks early
                qk_chain(wq_sb, xq_t, qh_sb, 0)
                qk_chain(wk_sb, xk_t, kh_sb, 0)
                for s4 in range(SQ_T // SK_T):
                    v_chain(s4)
                qk_chain(wq_sb, xq_t, qh_sb, 1)
                qk_chain(wk_sb, xk_t, kh_sb, 1)

            load_slab(0, split=True)
            # scalar HWDGE ring order: xk0 first (needed immediately), then
            # the later-needed weights
            nc.scalar.dma_start(wv_sb[:], wv)
            load_slab(1)
            nc.scalar.dma_start(wo_sb[:], wo)
            nc.scalar.dma_start(keep_sb[:], kp)
            warm_pe(12)  # span the initial DMA wait at full PE clock
            proj_slab(0)

            for t in range(NSQ):
                # keep projections one q-tile ahead of the attention stream
                if t + 2 < NSQ:
                    load_slab(t + 2)
                if t + 1 < NSQ:
                    proj_slab(t + 1)

                # ---- attention for q-tile j = t ----
                # High priority: the exp-gated attention chain must win the
                # scheduler heap over the (already-emitted) next-slab
                # projections, which then fill PE gaps instead of preempting.
                prio = tc.high_priority(offset=600)
                prio.__enter__()
                j = t
                jsl = bass.ts(j, SQ_T)
                kept = [i for i in range(NSK) if cls[(i, j)] != "skip"]
                for hp in range(2):
                    pv0 = pv_ps.tile([DH + 1, SQ_T], f32, tag="pv")
                    pv1 = pv_ps.tile([DH + 1, SQ_T], f32, tag="pv")
                    for n, i in enumerate(kept):
                        isl = bass.ts(i, SK_T)
                        uid, c0, c1, mrange = cls[(i, j)]
                        qsl = bass.ds(j * SQ_T + c0, c1 - c0)
                        sc = sc_ps.tile([128, 2, SQ_T], f32, tag="sc")
                        nc.tensor.matmul(sc[:, 0, c0:c1], kh_sb[:, hp, 0, isl],
                                         qh_sb[:, hp, qsl], start=True,
                                         stop=True)
                        nc.tensor.matmul(sc[:, 1, c0:c1], kh_sb[:, hp, 1, isl],
                                         qh_sb[:, hp, qsl], start=True,
                                         stop=True)
                        pr = prpool.tile([128, 2, SQ_T], bf, tag="probs")
                        nc.scalar.activation(pr[:, :, c0:c1], sc[:, :, c0:c1],
                                             EXP, scale=0.125)
                        if uid != "full":
                            m0, m1 = mrange
                            off = int(pat_off[uid])
                            nc.vector.tensor_mul(
                                pr[:, :, m0:m1], pr[:, :, m0:m1],
                                keep_sb[:, :, off:off + (m1 - m0)])
                        nc.tensor.matmul(pv0[:, c0:c1],
                                         vh_sb[:, i, 2 * hp + 0, :],
                                         pr[:, 0, c0:c1], start=(n == 0),
                                         stop=(n == len(kept) - 1))
                        nc.tensor.matmul(pv1[:, c0:c1],
                                         vh_sb[:, i, 2 * hp + 1, :],
                                         pr[:, 1, c0:c1], start=(n == 0),
                                         stop=(n == len(kept) - 1))
                    if j == NSQ - 1 and hp == 1:
                        warm_pe(10)  # keep PE clock up through the final chain
                    # evacuate psum: unnormalized att + denominators
                    nc.vector.tensor_copy(att_sb[0:64, hp, jsl], pv0[0:64, :])
                    nc.vector.tensor_copy(att_sb[64:128, hp, jsl], pv1[0:64, :])
                    p0 = 64 * hp
                    # denominator staging on ScalarE (it stalls here anyway)
                    nc.scalar.copy(l4[p0:p0 + 1, :], pv0[DH:DH + 1, :])
                    nc.scalar.copy(l4[p0 + 32:p0 + 33, :], pv1[DH:DH + 1, :])
                    # deferred normalization, per head-pair so hp0's chain
                    # overlaps hp1's attention: r = 1/l, then K=1 broadcast
                    # matmuls and an in-place multiply reading PSUM
                    nc.vector.reciprocal_approx_fast(r4f[:], l4[:])
                    nc.vector.tensor_copy(r4[:], r4f[:])
                    rb = pp_ps.tile([128, SQ_T], f32, tag="pp")
                    pe, po_ = 64 * hp, 64 * hp + 32
                    nc.tensor.matmul(rb[0:64, :], ones_sb[pe:pe + 1, 0:64],
                                     r4[pe:pe + 1, :], start=True, stop=True,
                                     tile_position=(pe, 0))
                    nc.tensor.matmul(rb[64:128, :], ones_sb[po_:po_ + 1, 64:128],
                                     r4[po_:po_ + 1, :], start=True, stop=True,
                                     tile_position=(po_, 64))
                    nc.vector.tensor_mul(att_sb[:, hp, jsl], att_sb[:, hp, jsl],
                                         rb[:])
                # ---- output projection for this q-tile ----
                for t4 in range(SQ_T // 128):
                    r_ = j * (SQ_T // 128) + t4
                    tsl = bass.ds(j * SQ_T + t4 * 128, 128)
                    po0 = pp_ps.tile([128, SQ_T], f32, tag="pp")
                    po1 = pp_ps.tile([128, SQ_T], f32, tag="pp")
                    for hp in range(2):
                        nc.tensor.matmul(po0[:], att_sb[:, hp, tsl],
                                         wo_sb[:, hp, 0:512],
                                         start=(hp == 0), stop=(hp == 1))
                        nc.tensor.matmul(po1[:], att_sb[:, hp, tsl],
                                         wo_sb[:, hp, 512:1024],
                                         start=(hp == 0), stop=(hp == 1))
                    ost = work.tile([128, D], bf, tag="ost")
                    nc.vector.tensor_copy(ost[:, 0:512], po0[:])
                    nc.vector.tensor_copy(ost[:, 512:1024], po1[:])
                    nc.sync.dma_start(out[r_], ost[:])
                prio.__exit__(None, None, None)

    nc.compile()
    return nc


def _get_nc(mask):
    key = hash(np.asarray(mask, dtype=bool).tobytes())
    if key not in _BUILT:
        cls, patterns = _classify(mask)
        widths = [p.shape[1] for p in patterns]
        _BUILT[key] = (_build(cls, widths), cls, patterns)
    return _BUILT[key]


def _pack_x(a):
    """[S, D] f32 -> [NSQ, 128, NCH, SQ_T] bf16 with x[t,p,c,s] = a[t*512+s, c*128+p]."""
    r = a.astype(BF16).reshape(NSQ, SQ_T, NCH, 128).transpose(0, 3, 2, 1)
    return np.ascontiguousarray(r)


def _pack_w(w):
    """[FPG, D] slice -> [128, NCH, FPG] bf16 with out[p,c,f] = w[f, c*128+p]."""
    r = w.T.astype(BF16).reshape(NCH, 128, FPG).transpose(1, 0, 2)
    return np.ascontiguousarray(r)


def _pack_keep(patterns):
    if not patterns:
        return np.zeros((128, 2, 1), dtype=BF16)
    cat = np.concatenate(patterns, axis=1)  # [128, KW]
    return np.ascontiguousarray(np.broadcast_to(cat[:, None, :],
                                                (128, 2, cat.shape[1])))


def _kernel_impl(q, k, v, attn_mask, Wq, Wk, Wv, Wo, trace=False):
    q = np.asarray(q, dtype=np.float32)
    k = np.asarray(k, dtype=np.float32)
    v = np.asarray(v, dtype=np.float32)
    nc, cls, patterns = _get_nc(attn_mask)
    keep_packed = _pack_keep(patterns)

    xt = {}
    for b in range(B):
        xt[("q", b)] = _pack_x(q[b])
        xt[("k", b)] = _pack_x(k[b])
        xt[("v", b)] = _pack_x(v[b])
    wslices = {}
    for g in range(GROUPS):
        fsl = slice(g * FPG, (g + 1) * FPG)
        wslices[("wq", g)] = _pack_w(Wq[fsl, :])
        wslices[("wk", g)] = _pack_w(Wk[fsl, :])
        wslices[("wv", g)] = _pack_w(Wv[fsl, :])
        # wo: [128, 2, D] with out[p,h,o] = Wo[o, g*256 + h*128 + p]
        woT = Wo[:, fsl].T.astype(BF16)  # [256, 1024]
        wslices[("wo", g)] = np.ascontiguousarray(
            woT.reshape(2, 128, D).transpose(1, 0, 2))

    in_maps = []
    for core in range(NCORES):
        b, g = core // GROUPS, core % GROUPS
        in_maps.append({
            "xqt": xt[("q", b)], "xkt": xt[("k", b)], "xvt": xt[("v", b)],
            "wqt": wslices[("wq", g)], "wkt": wslices[("wk", g)],
            "wvt": wslices[("wv", g)], "wot": wslices[("wo", g)],
            "keep": keep_packed,
        })

    res = bass_utils.run_bass_kernel_spmd(
        nc, in_maps, core_ids=list(range(NCORES)), trace=trace)

    out = np.zeros((B, S, D), dtype=np.float32)
    for core in range(NCORES):
        r = np.asarray(res.results[core]["out"]).astype(np.float32)
        out[core // GROUPS] += r.reshape(S, D)
    return out, res


def kernel(q, k, v, attn_mask, Wq, Wk, Wv, Wo):
    out, _ = _kernel_impl(q, k, v, attn_mask, Wq, Wk, Wv, Wo)
    return out
